# revision 10
# baseline (speedup 1.0000x reference)
"""Trainium2 Bass kernel for nn_Contraction — flipped orientation, v2.

Per node b (one node = 128 channels = one partition-block):
  out1[c, (w,x2,v)] = sum_ki t4[(k,i), c] * U3cat[(k,i), (w,x2,v)]   (PE)
  out2[c, (w,x2)]   = sum_v out1[c, (w,x2,v)] * x[c, v]   (Pool mul + DVE X-reduce)
  out3[c, w]        = sum_x2 out2[c, (w,x2)] * x[c, x2]   (Pool mul + DVE X-reduce)
  final[b, c, w]    = out3 + q  where q = wn1*(U1 . x) is host-precomputed
                      and added during the end-phase transpose copies.

PE does ONLY the U3 contraction (6 matmuls/node, 2304 output-el-cycles =
dense-FLOP floor) plus 3 end transposes. No selector matmuls, no xrep.
The U2 term rides rows 112:117 of K-chunk 2; the U1 term is folded into
the end-phase PSUM->SBUF adds (q shipped from host).

Engine split: ACT copies PSUM out1 -> SBUF bf16 (one copy per node from a
padded 2-bank PSUM tile); Pool (GPSIMD) does the bf16 elementwise muls
(cannot read PSUM); DVE does the segmented X-axis add-reduces (f32 out).
Elementwise ops are batched across 2 nodes (v-stage) / 8 nodes (x2-stage)
to amortize per-instruction overheads.

Sharding: data-parallel over nodes b across 8 cores (128 nodes/core).
"""

import sys

if "/opt/trn_rl_repo" not in sys.path:
    sys.path.insert(0, "/opt/trn_rl_repo")

import numpy as np
import ml_dtypes

import concourse.bass as bass
import concourse.mybir as mybir
import concourse.tile as tile
from concourse.masks import make_identity

dt = mybir.dt

B, C, ELL, EQ, E = 1024, 128, 16, 3, 10
P3, P2, P1 = 23, 5, 1
N_CORES = 8
BS = B // N_CORES          # nodes per core (128)
WXV = EQ * ELL * ELL       # 768
WX2 = EQ * ELL             # 48
KCH = (128, 128, 112 + P2) # K chunks (chunk2: 112 U3-rows + 5 U2-rows)
CH = 16                    # nodes per DMA chunk
NCH = BS // CH             # chunks per core (8)
FC = CH * C                # chunk free width (2048)
HV = WXV // 2              # matmul N half (384)

_f32 = dt.float32
_bf16 = dt.bfloat16
_bf = ml_dtypes.bfloat16
_mult = mybir.AluOpType.mult
_add = mybir.AluOpType.add
_AX = mybir.AxisListType.X


def _build_program():
    nc = bass.Bass("TRN2", target_bir_lowering=False, debug=False)

    trep_d = nc.dram_tensor("trep", [3, 128, BS, C], _bf16, kind="ExternalInput")
    xf_d = nc.dram_tensor("xf", [C, BS, ELL], _bf16, kind="ExternalInput")
    q_d = nc.dram_tensor("q", [BS, C * EQ], _bf16, kind="ExternalInput")
    u3cat_d = nc.dram_tensor("u3cat", [3, 128, WXV], _bf16, kind="ExternalInput")
    out_d = nc.dram_tensor("out", [BS, C * EQ], _f32, kind="ExternalOutput")

    with tile.TileContext(nc) as tc:
        with tc.tile_pool(name="const", bufs=1) as cpool:
            u3sb = cpool.tile([128, 3, WXV], _bf16)
            nc.sync.dma_start(out=u3sb[:], in_=u3cat_d[:].rearrange("j p f -> p j f"))
            xfsb = cpool.tile([C, BS, ELL], _bf16)
            nc.sync.dma_start(out=xfsb[:], in_=xf_d[:])
            qsb = cpool.tile([BS, C * EQ], _bf16)
            nc.sync.dma_start(out=qsb[:], in_=q_d[:])
            outsb = cpool.tile([C, BS * EQ], _f32)    # [c, (b, w)] staging

            with tc.tile_pool(name="work", bufs=2) as pool, \
                 tc.tile_pool(name="pwork", bufs=3) as ppool, \
                 tc.tile_pool(name="owork", bufs=2) as opool, \
                 tc.tile_pool(name="ps1", bufs=4, space="PSUM") as psb:
                for ci in range(NCH):
                    bsl = slice(ci * CH, (ci + 1) * CH)
                    tch = pool.tile([128, 3, FC], _bf16, tag="tch")
                    nc.sync.dma_start(
                        out=tch[:], in_=trep_d[:, :, bsl].rearrange("j p b c -> p j b c")
                    )
                    for oi in range(CH // 8):      # 8-node blocks
                        ob = ci * CH + oi * 8
                        out2b = opool.tile([C, 8, WX2], _f32, tag="out2b")
                        for pi in range(4):        # node pairs
                            np0 = ob + 2 * pi
                            o1b = ppool.tile([C, 2, WXV], _bf16, tag="o1b")
                            for u in range(2):     # nodes in pair
                                n = np0 + u
                                nsl = slice((n - ci * CH) * C, (n - ci * CH + 1) * C)
                                # padded PSUM tile: each N=384 half in its own bank
                                ps1 = psb.tile([C, 2, 512], _f32, tag="ps1")
                                for h in range(2):
                                    for j in range(3):
                                        nc.tensor.matmul(
                                            ps1[:, h, :HV],
                                            tch[: KCH[j], j, nsl],
                                            u3sb[: KCH[j], j, h * HV : (h + 1) * HV],
                                            start=(j == 0),
                                            stop=(j == 2),
                                        )
                                nc.scalar.copy(
                                    o1b[:, u, :].rearrange("p (h f) -> p h f", h=2),
                                    ps1[:, :, :HV],
                                )
                            # v-contraction for the pair; the mul is split
                            # Pool/DVE (Pool is ~2ns/el with broadcast
                            # operands, DVE ~1ns/el) to balance engines
                            xv2 = xfsb[:, np0 : np0 + 2, None, :]
                            m1 = ppool.tile([C, 2, WX2, ELL], _bf16, tag="m1")
                            o1v = o1b[:].rearrange("p u (a v) -> p u a v", v=ELL)
                            SP = 30
                            nc.gpsimd.tensor_mul(
                                m1[:, :, :SP, :],
                                o1v[:, :, :SP, :],
                                xv2.to_broadcast([C, 2, SP, ELL]),
                            )
                            nc.vector.tensor_mul(
                                m1[:, :, SP:, :],
                                o1v[:, :, SP:, :],
                                xv2.to_broadcast([C, 2, WX2 - SP, ELL]),
                            )
                            nc.vector.tensor_reduce(
                                out2b[:, 2 * pi : 2 * pi + 2, :], m1[:], _AX, _add
                            )
                        # x2-contraction for the 8-node block
                        xv8 = xfsb[:, ob : ob + 8, None, :]
                        m2 = opool.tile([C, 8, EQ, ELL], _bf16, tag="m2")
                        nc.gpsimd.tensor_mul(
                            m2[:],
                            out2b[:].rearrange("p u (w i) -> p u w i", i=ELL),
                            xv8.to_broadcast([C, 8, EQ, ELL]),
                        )
                        nc.vector.tensor_reduce(
                            outsb[:, ob * EQ : (ob + 8) * EQ], m2[:], _AX, _add
                        )

            # -------- end phase: transpose [c,(b,w)] -> [b,(c,w)], + q --------
            with tc.tile_pool(name="fin", bufs=2) as fpool, \
                 tc.tile_pool(name="ps_fin", bufs=2, space="PSUM") as psf:
                ident128 = cpool.tile([128, 128], _f32)
                make_identity(nc, ident128[:])

                finsb = fpool.tile([BS, C * EQ], _f32, tag="finsb")
                outsb_r = outsb[:].rearrange("c (b w) -> c b w", w=EQ)
                finsb_r = finsb[:].rearrange("b (c w) -> b c w", w=EQ)
                qsb_r = qsb[:].rearrange("b (c w) -> b c w", w=EQ)
                for w in range(EQ):
                    fin_ps = psf.tile([BS, C], _f32, tag="fin")
                    nc.tensor.transpose(fin_ps[:], outsb_r[:, :, w], ident128[:])
                    nc.vector.tensor_add(finsb_r[:, :, w], fin_ps[:], qsb_r[:, :, w])

                nc.sync.dma_start(out=out_d[:], in_=finsb[:])

    import bass_rust
    bass_rust.move_matmul_waits_to_ldweights(nc.m)
    bass_rust.generate_event_semaphores(nc)
    return nc


def _host_prep(x, y, U3, U2, U1, w_max, w2, w1):
    x = np.ascontiguousarray(x, dtype=np.float32)
    elem = np.argmax(y, axis=1)

    wn3 = w_max[elem]                       # [B, 23, C]
    wn1 = w1[elem][:, 0, :]                 # [B, C]

    # trep[j, p, b, c] = x[b, c, i(p)] * wn3[b, 8j + p//16, c]; chunk2 rows
    # 112:117 = wn2 (folded U2 contraction operand)
    trep = np.zeros((B, 3, 128, C), dtype=np.float32)
    wn3r = np.repeat(wn3, ELL, axis=1)      # [B, 368, C]
    xtile = np.tile(x.transpose(0, 2, 1), (1, P3, 1))  # [B, 368, C]
    trep.reshape(B, 384, C)[:, :368, :] = wn3r * xtile
    trep[:, 2, 112 : 112 + P2, :] = w2[elem]
    trep = np.ascontiguousarray(trep.transpose(1, 2, 0, 3)).astype(_bf)  # [3,128,B,C]

    xf = np.ascontiguousarray(x.transpose(1, 0, 2)).astype(_bf)   # [C, B, ELL]

    # q[b, c, w] = wn1[b,c] * sum_x2 U1[w,x2]*x[b,c,x2]  (U1 path, host)
    q = wn1[:, :, None] * np.einsum("wi,bci->bcw", U1[:, :, 0], x)
    q = q.reshape(B, C * EQ).astype(_bf)

    # U3cat: [(k,i), (w, x2, v)] chunks of 128; chunk2 rows 112:117 = U2
    u3k = U3.transpose(4, 3, 0, 1, 2).reshape(ELL * P3, WXV)
    u2k = U2.transpose(3, 0, 1, 2).reshape(P2, WXV)
    u3cat = np.zeros((3, 128, WXV), dtype=np.float32)
    u3cat[0] = u3k[0:128]
    u3cat[1] = u3k[128:256]
    u3cat[2, 0:112] = u3k[256:368]
    u3cat[2, 112 : 112 + P2] = u2k
    u3cat = u3cat.astype(_bf)

    shared = {"u3cat": u3cat}

    def per_core(ci):
        s = slice(ci * BS, (ci + 1) * BS)
        m = {
            "trep": np.ascontiguousarray(trep[:, :, s]),
            "xf": np.ascontiguousarray(xf[:, s]),
            "q": np.ascontiguousarray(q[s]),
        }
        m.update(shared)
        return m

    return per_core


_PROGRAM_CACHE = {}


def kernel(**inputs) -> np.ndarray:
    from concourse.bass_utils import run_bass_kernel_spmd

    per_core = _host_prep(
        np.asarray(inputs["x"]), np.asarray(inputs["y"]),
        np.asarray(inputs["U3"]), np.asarray(inputs["U2"]),
        np.asarray(inputs["U1"]), np.asarray(inputs["w_max"]),
        np.asarray(inputs["w2"]), np.asarray(inputs["w1"]),
    )

    if "nc" not in _PROGRAM_CACHE:
        _PROGRAM_CACHE["nc"] = _build_program()
    nc = _PROGRAM_CACHE["nc"]

    in_maps = [per_core(ci) for ci in range(N_CORES)]
    res = run_bass_kernel_spmd(nc, in_maps, core_ids=list(range(N_CORES)))
    out = np.concatenate([r["out"] for r in res.results], axis=0)
    return out.astype(np.float32)


if __name__ == "__main__":
    from concourse.bass_interp import CoreSim

    rng = np.random.default_rng(0)
    x = rng.standard_normal((B, C, ELL)).astype(np.float32)
    elem = rng.integers(0, E, size=B)
    y = np.eye(E, dtype=np.float32)[elem]
    U3 = (rng.standard_normal((EQ, ELL, ELL, ELL, P3)) * 0.1).astype(np.float32)
    U2 = (rng.standard_normal((EQ, ELL, ELL, P2)) * 0.1).astype(np.float32)
    U1 = (rng.standard_normal((EQ, ELL, P1)) * 0.1).astype(np.float32)
    w_max = (rng.standard_normal((E, P3, C)) / P3).astype(np.float32)
    w2 = (rng.standard_normal((E, P2, C)) / P2).astype(np.float32)
    w1 = (rng.standard_normal((E, P1, C)) / P1).astype(np.float32)

    per_core = _host_prep(x, y, U3, U2, U1, w_max, w2, w1)
    nc = _build_program()
    sim = CoreSim(nc)
    m = per_core(0)
    for k, v in m.items():
        sim.tensor(k)[:] = v
    sim.simulate(check_with_hw=False, trace_hw=False)
    got = np.array(sim.tensor("out"))

    def ref_np(x, y, U3, U2, U1, w_max, w2, w1):
        wn3 = np.einsum("be,ekc->bkc", y, w_max)
        t = np.einsum("bkc,bci->bcik", wn3, x)
        out = np.einsum("wxvik,bcik->bcwxv", U3, t)
        wn2 = np.einsum("be,ekc->bkc", y, w2)
        c2 = np.einsum("wxvk,bkc->bcwxv", U2, wn2) + out
        out = np.einsum("bcwxi,bci->bcwx", c2, x)
        wn1 = np.einsum("be,ekc->bkc", y, w1)
        c1 = np.einsum("wxk,bkc->bcwx", U1, wn1) + out
        out = np.einsum("bcwi,bci->bcw", c1, x)
        return out.reshape(out.shape[0], -1)

    want = ref_np(x[:BS], y[:BS], U3, U2, U1, w_max, w2, w1)
    err = np.abs(got - want).max() / (np.abs(want).max() + 1e-30)
    print(f"CoreSim vs numpy rel err: {err:.3e}")
    assert err < 2e-2, "FAIL"
    print("SIM PASS")


# revision 18
# speedup vs baseline: 1.0089x; 1.0089x over previous
"""Trainium2 Bass kernel for nn_Contraction (MACE-style CG contraction).

Math (per node b, channel c):
  wn3 = w_max[elem_b]  (23,C) ; wn2 = w2[elem_b] (5,C) ; wn1 = w1[elem_b] (1,C)
  t[(k,i)]   = wn3[k,c] * x[b,c,i]                        (368)
  c2[wxv]    = sum_ik U3[w,x2,v,i,k] t[(k,i)] + sum_k2 U2[w,x2,v,k2] wn2[k2,c]
  out2[wx2]  = sum_v c2[(w,x2,v)] * x[b,c,v]
  out3[w]    = sum_x2 (out2[(w,x2)] + U1[w,x2,0]*wn1[0,c]) * x[b,c,x2]
  out[b, c*3+w] = out3[w]

Device mapping (per core, Bs=128 nodes, groups of G=4 nodes, F=G*C=512):
  - all matmul operands bf16 (1 cyc/row on PE vs ~4 for fp32r); PSUM acc f32.
  - main matmul: lhsT = U3cat chunks (stationary), rhs = t4 [(k,i)-chunks,
    (b4,c)] streamed; out1T [(wxv) 6 x 128p, (b4,c)] accumulated in PSUM
    (6 banks). K chunks 128/128/117 (U2 folded as rows 112:117 of chunk 2).
  - t4 products computed on host (free), shipped bf16.
  - v-contraction: m6 = out1T * xrep elementwise, split across DVE (half 0)
    and Pool (half 1); then 6 accumulating selector matmuls (PE) + U1*wn1
    row -> c1 [48, F] in PSUM.
  - x2-contraction: m9e = c1_ps * xrep[:48] (DVE), 4 per-node matmuls with
    sel9 -> out3T [c, (b,w)].
  - end phase: 3 [C,BS] -> [BS,C] transposes into (b, c, w) layout, single
    contiguous DMA out (f32).
  - DMA: streamed tensors are loaded in 4-group chunks with host layouts
    arranged so each partition line is one contiguous 4KB block.

Sharding: data-parallel over nodes b across 8 cores (128 nodes/core).
Host prep (numpy): elem = argmax(y), per-element weight gather, t4 product,
layout packs, bf16 casts. All device FLOPs per the mapping above.
"""

import sys

if "/opt/trn_rl_repo" not in sys.path:
    sys.path.insert(0, "/opt/trn_rl_repo")

import numpy as np
import ml_dtypes

import concourse.bass as bass
import concourse.mybir as mybir
import concourse.tile as tile
from concourse.masks import make_identity

dt = mybir.dt

# problem constants (hardcoded per contract)
B, C, ELL, EQ, E = 1024, 128, 16, 3, 10
P3, P2, P1 = 23, 5, 1
N_CORES = 8
BS = B // N_CORES          # nodes per core
G = 4                      # nodes per group
NG = BS // G               # groups per core
F = G * C                  # streamed free dim (b4, c) = 512
WXV = EQ * ELL * ELL       # 768
WX2 = EQ * ELL             # 48
KTOT = ELL * P3 + P2       # 373
KCH = (128, 128, 112 + P2) # K chunks (chunk2: 112 U3-rows + 5 U2-rows)
CH = 4                     # groups per DMA chunk
NCH = NG // CH             # chunks per core
FC = CH * F                # chunk free width (2048)

_f32 = dt.float32
_bf16 = dt.bfloat16
_bf = ml_dtypes.bfloat16


def _build_program():
    """Build the per-core Bass program (identical across cores)."""
    nc = bass.Bass("TRN2", target_bir_lowering=False, debug=False)

    xrep_d = nc.dram_tensor("xrep", [128, BS * C], _bf16, kind="ExternalInput")
    trep_d = nc.dram_tensor("trep", [3, 128, BS, C], _bf16, kind="ExternalInput")
    q_d = nc.dram_tensor("q", [BS, C * EQ], _bf16, kind="ExternalInput")
    u3cat_d = nc.dram_tensor("u3cat", [3, 128, WXV], _bf16, kind="ExternalInput")
    sel6_d = nc.dram_tensor("sel6", [128, 6, WX2], _bf16, kind="ExternalInput")
    sel9_d = nc.dram_tensor("sel9", [WX2, EQ], _bf16, kind="ExternalInput")
    out_d = nc.dram_tensor("out", [BS, C * EQ], _f32, kind="ExternalOutput")

    with tile.TileContext(nc) as tc:
        with tc.tile_pool(name="const", bufs=1) as cpool:
            u3sb = cpool.tile([128, 3, WXV], _bf16)
            nc.sync.dma_start(out=u3sb[:], in_=u3cat_d[:].rearrange("j p f -> p j f"))
            sel6sb = cpool.tile([128, 6, WX2], _bf16)
            nc.sync.dma_start(out=sel6sb[:], in_=sel6_d[:])
            sel9sb = cpool.tile([WX2, EQ], _bf16)
            nc.sync.dma_start(out=sel9sb[:], in_=sel9_d[:])
            qsb = cpool.tile([BS, C * EQ], _bf16)     # host U1-path term
            nc.sync.dma_start(out=qsb[:], in_=q_d[:])
            outsb = cpool.tile([C, BS * EQ], _bf16)   # [c, (b, w)] staging

            # ---------------- main loop over chunks of CH groups ----------
            with tc.tile_pool(name="work", bufs=2) as pool, \
                 tc.tile_pool(name="mwork", bufs=2) as mpool, \
                 tc.tile_pool(name="ps_big", bufs=1, space="PSUM") as psb, \
                 tc.tile_pool(name="ps_c1", bufs=1, space="PSUM") as psc, \
                 tc.tile_pool(name="ps_o3", bufs=1, space="PSUM") as pso:
                for ci in range(NCH):
                    bsl = slice(ci * CH * G, (ci + 1) * CH * G)

                    # xrep chunk: [p, (b, c)] contiguous 4KB lines
                    xch = pool.tile([128, FC], _bf16, tag="xch")
                    nc.sync.dma_start(
                        out=xch[:], in_=xrep_d[:, ci * FC : (ci + 1) * FC]
                    )
                    # t4 chunk: [p, j, (b, c)]; per (p, j) one contiguous 4KB
                    tch = pool.tile([128, 3, FC], _bf16, tag="tch")
                    nc.sync.dma_start(
                        out=tch[:], in_=trep_d[:, :, bsl].rearrange("j p b c -> p j b c")
                    )

                    for gi in range(CH):
                        g = ci * CH + gi
                        fsl = slice(gi * F, (gi + 1) * F)
                        xrep = xch[:, fsl]

                        # main matmuls (two 3-bank PSUM halves) + m6 =
                        # out1T * x_v (v = p % 16) per half; DVE does half
                        # 0, Pool half 1 to split the elementwise load.
                        m6 = mpool.tile([128, 6, F], _bf16, tag="m6")
                        halves = []
                        for h in range(2):
                            ph = psb.tile([128, 3, F], _f32, tag=f"out1{h}")
                            halves.append(ph)
                            for mm in range(3):
                                m = 3 * h + mm
                                for j in range(3):
                                    k = KCH[j]
                                    nc.tensor.matmul(
                                        ph[:, mm, :],
                                        u3sb[:k, j, 128 * m : 128 * (m + 1)],
                                        tch[:k, j, fsl],
                                        start=(j == 0),
                                        stop=(j == 2),
                                    )
                            nc.vector.tensor_mul(
                                m6[:, 3 * h : 3 * (h + 1), :],
                                ph[:],
                                xrep[:, None, :].to_broadcast([128, 3, F]),
                            )

                        c1_ps = psc.tile([WX2, F], _f32, tag="c1")

                        # 6 accumulating selector matmuls -> c1 (U1 path
                        # is folded into the end-phase q-add instead)
                        for m in range(6):
                            nc.tensor.matmul(
                                c1_ps[:],
                                sel6sb[:, m, :],
                                m6[:, m, :],
                                start=(m == 0),
                                stop=(m == 5),
                            )

                        m9e = mpool.tile([WX2, F], _bf16, tag="m9e")
                        nc.vector.tensor_mul(m9e[:], c1_ps[:], xrep[:WX2, :])

                        # final contraction per node: lhsT = m9e b-slice
                        # [48, C], rhs = sel9 [48, 3] -> out [c, 3]
                        o3_ps = pso.tile([C, G, EQ], _f32, tag="o3")
                        for b in range(G):
                            nc.tensor.matmul(
                                o3_ps[:, b, :],
                                m9e[:, C * b : C * (b + 1)],
                                sel9sb[:],
                                start=True,
                                stop=True,
                            )
                        nc.scalar.copy(
                            outsb[:, g * G * EQ : (g + 1) * G * EQ], o3_ps[:]
                        )

            # ---------------- end phase: layout transform ----------------
            with tc.tile_pool(name="fin", bufs=2) as fpool, \
                 tc.tile_pool(name="ps_fin", bufs=2, space="PSUM") as psf:
                ident128 = cpool.tile([128, 128], _bf16)
                make_identity(nc, ident128[:])

                # [c, (b, w)] -> [b, (c, w)] via 3 big transposes
                finsb = fpool.tile([BS, C * EQ], _f32, tag="finsb")
                outsb_r = outsb[:].rearrange("c (b w) -> c b w", w=EQ)
                finsb_r = finsb[:].rearrange("b (c w) -> b c w", w=EQ)
                qsb_r = qsb[:].rearrange("b (c w) -> b c w", w=EQ)
                for w in range(EQ):
                    fin_ps = psf.tile([BS, C], _bf16, tag="fin")
                    nc.tensor.transpose(fin_ps[:], outsb_r[:, :, w], ident128[:])
                    nc.vector.tensor_add(finsb_r[:, :, w], fin_ps[:], qsb_r[:, :, w])

                nc.sync.dma_start(out=out_d[:], in_=finsb[:])

    # Walrus codegen allows at most one sync-wait per instruction; Tile can
    # emit more. Split them exactly as Bacc.compile does.
    import bass_rust
    bass_rust.move_matmul_waits_to_ldweights(nc.m)
    bass_rust.generate_event_semaphores(nc)
    return nc


def _host_prep(x, y, U3, U2, U1, w_max, w2, w1):
    """Numpy-side input prep: gather per-element weights, t4 product,
    layout packs, bf16 casts. Returns per_core(ci) -> input map."""
    x = np.ascontiguousarray(x, dtype=np.float32)
    elem = np.argmax(y, axis=1)

    wn3 = w_max[elem]                       # [B, 23, C]
    wn1 = w1[elem][:, 0, :]                 # [B, C]

    # trep[j, p, b, c] = x[b, c, i(p)] * wn3[b, 8j + p//16, c]; chunk2 rows
    # 112:117 = wn2 (folded U2 contraction operand)
    trep = np.zeros((B, 3, 128, C), dtype=np.float32)
    wn3r = np.repeat(wn3, ELL, axis=1)      # [B, 368, C]
    xtile = np.tile(x.transpose(0, 2, 1), (1, P3, 1))  # [B, 368, C]
    trep.reshape(B, 384, C)[:, :368, :] = wn3r * xtile
    trep[:, 2, 112 : 112 + P2, :] = w2[elem]
    trep = np.ascontiguousarray(trep.transpose(1, 2, 0, 3)).astype(_bf)  # [3,128,B,C]

    # xrep[p, b, c] = x[b, c, p % 16]  (p-major so DMA lines are contiguous)
    xT = np.ascontiguousarray(x.transpose(2, 0, 1))  # [16, B, C]
    xrep = np.tile(xT, (8, 1, 1)).reshape(128, B * C).astype(_bf)  # [128, B*C]

    # q[b, c, w] = wn1[b,c] * sum_x2 U1[w,x2]*x[b,c,x2]  (U1 path, host)
    q = (wn1[:, :, None] * np.einsum("wi,bci->bcw", U1[:, :, 0], x))
    q = q.reshape(B, C * EQ).astype(_bf)

    # U3cat: [k, i, (w, x2, v)] chunks of 128; chunk2 rows 112:117 = U2
    u3k = U3.transpose(4, 3, 0, 1, 2).reshape(ELL * P3, WXV)  # [(k,i), wxv]
    u2k = U2.transpose(3, 0, 1, 2).reshape(P2, WXV)
    u3cat = np.zeros((3, 128, WXV), dtype=np.float32)
    u3cat[0] = u3k[0:128]
    u3cat[1] = u3k[128:256]
    u3cat[2, 0:112] = u3k[256:368]
    u3cat[2, 112 : 112 + P2] = u2k
    u3cat = u3cat.astype(_bf)

    sel6 = np.zeros((128, 6, WX2), dtype=_bf)
    for m in range(6):
        for p in range(128):
            sel6[p, m, 8 * m + p // 16] = 1.0

    sel9 = np.zeros((WX2, EQ), dtype=_bf)
    for p in range(WX2):
        sel9[p, p // 16] = 1.0

    shared = {"u3cat": u3cat, "sel6": sel6, "sel9": sel9}

    def per_core(ci):
        s = slice(ci * BS, (ci + 1) * BS)
        cs = slice(ci * BS * C, (ci + 1) * BS * C)
        m = {
            "xrep": np.ascontiguousarray(xrep[:, cs]),
            "trep": np.ascontiguousarray(trep[:, :, s]),
            "q": np.ascontiguousarray(q[s]),
        }
        m.update(shared)
        return m

    return per_core


_PROGRAM_CACHE = {}


def kernel(**inputs) -> np.ndarray:
    from concourse.bass_utils import run_bass_kernel_spmd

    per_core = _host_prep(
        np.asarray(inputs["x"]), np.asarray(inputs["y"]),
        np.asarray(inputs["U3"]), np.asarray(inputs["U2"]),
        np.asarray(inputs["U1"]), np.asarray(inputs["w_max"]),
        np.asarray(inputs["w2"]), np.asarray(inputs["w1"]),
    )

    if "nc" not in _PROGRAM_CACHE:
        _PROGRAM_CACHE["nc"] = _build_program()
    nc = _PROGRAM_CACHE["nc"]

    in_maps = [per_core(ci) for ci in range(N_CORES)]
    res = run_bass_kernel_spmd(nc, in_maps, core_ids=list(range(N_CORES)))
    out = np.concatenate([r["out"] for r in res.results], axis=0)
    return out.astype(np.float32)


if __name__ == "__main__":
    # smoke test in CoreSim on core 0's shard
    from concourse.bass_interp import CoreSim

    rng = np.random.default_rng(0)
    x = rng.standard_normal((B, C, ELL)).astype(np.float32)
    elem = rng.integers(0, E, size=B)
    y = np.eye(E, dtype=np.float32)[elem]
    U3 = (rng.standard_normal((EQ, ELL, ELL, ELL, P3)) * 0.1).astype(np.float32)
    U2 = (rng.standard_normal((EQ, ELL, ELL, P2)) * 0.1).astype(np.float32)
    U1 = (rng.standard_normal((EQ, ELL, P1)) * 0.1).astype(np.float32)
    w_max = (rng.standard_normal((E, P3, C)) / P3).astype(np.float32)
    w2 = (rng.standard_normal((E, P2, C)) / P2).astype(np.float32)
    w1 = (rng.standard_normal((E, P1, C)) / P1).astype(np.float32)

    per_core = _host_prep(x, y, U3, U2, U1, w_max, w2, w1)
    nc = _build_program()
    sim = CoreSim(nc)
    m = per_core(0)
    for k, v in m.items():
        sim.tensor(k)[:] = v
    sim.simulate(check_with_hw=False, trace_hw=False)
    got = np.array(sim.tensor("out"))

    # numpy reference for core 0 shard
    def ref_np(x, y, U3, U2, U1, w_max, w2, w1):
        wn3 = np.einsum("be,ekc->bkc", y, w_max)
        t = np.einsum("bkc,bci->bcik", wn3, x)
        out = np.einsum("wxvik,bcik->bcwxv", U3, t)
        wn2 = np.einsum("be,ekc->bkc", y, w2)
        c2 = np.einsum("wxvk,bkc->bcwxv", U2, wn2) + out
        out = np.einsum("bcwxi,bci->bcwx", c2, x)
        wn1 = np.einsum("be,ekc->bkc", y, w1)
        c1 = np.einsum("wxk,bkc->bcwx", U1, wn1) + out
        out = np.einsum("bcwi,bci->bcw", c1, x)
        return out.reshape(out.shape[0], -1)

    want = ref_np(x[:BS], y[:BS], U3, U2, U1, w_max, w2, w1)
    err = np.abs(got - want).max() / (np.abs(want).max() + 1e-30)
    print(f"CoreSim vs numpy rel err: {err:.3e}")
    assert err < 2e-2, "FAIL"
    print("SIM PASS")
